# revision 6
# baseline (speedup 1.0000x reference)
"""Fused GNN-message-passing kernel for Trainium2 (8 NeuronCores, data-parallel).

reference math: for each of B=1024 graphs with 32 nodes, all 496 node pairs
(i<j) in both orderings: features = [onehot_i, onehot_j, PE(pos_j-pos_i),
PE(qinv(q_i) x q_j)] -> 146-dim -> MLP(128,128,128,1) -> 0.003*tanh -> mean
over the two orderings -> out [B, 496].

Structure exploited on device:
- dir-1 features equal dir-0 features up to cos-even / sin-odd symmetry and a
  type-block swap, so trig features are computed ONCE and the two directions
  use different host-precomputed W0 operands.
- pair seeds (pd, lq, onehots) are built token-major with elementwise engines,
  PE-transposed to feature-major, and expanded to angle rows by one sparse
  matmul whose coefficients also fold in the 2^i*pi frequency scaling and the
  1/(2*pi) turns conversion.
- sin/cos via range reduction (magic-number rounding to turns) + ACT Sin.
- biases: b0 rides an all-ones seed row; b1/b2 fused into relu copies; b3 in
  the tanh activation bias.
"""
import numpy as np
import concourse.bass as bass
import concourse.mybir as mybir
import concourse.tile as tile
import concourse.bacc as bacc
from concourse.bass_utils import run_bass_kernel_spmd

F32 = mybir.dt.float32
AF = mybir.ActivationFunctionType
ALU = mybir.AluOpType
MS = bass.MemorySpace

B, NN, FDIM, EMB = 1024, 32, 128, 10
NCORES, BC = 8, B // 8          # 128 batches per core
NPAIR = NN * (NN - 1) // 2      # 496
PPB = 8                         # pairs per block / chunk
NBLK = NPAIR // PPB             # 62 chunks
NH = 512                        # MLP half-chunk width (4 pairs x 128 b)
N1 = PPB * BC                   # 1024 tokens per chunk (dir-0 only)
MAGIC = float(np.float32(1.5 * 2 ** 23))
TWO_PI = float(2 * np.pi)
PEN = 0.003


def _host_operands(W0, b0, W3):
    cos_rows = [6 + (r // 3) * 6 + (r % 3) for r in range(30)] + \
               [66 + (rr // 4) * 8 + (rr % 4) for rr in range(40)]
    sin_rows = [6 + (r // 3) * 6 + 3 + (r % 3) for r in range(30)] + \
               [66 + (rr // 4) * 8 + 4 + (rr % 4) for rr in range(40)]
    w0cos0 = W0[cos_rows]                       # [70,128]
    w0sin0 = W0[sin_rows]
    sgn = np.ones(70, np.float32)
    sgn[:30] = -1.0                             # pd flips sign in dir 1
    for rr in range(40):                        # lq vector part flips, w keeps
        sgn[30 + rr] = -1.0 if (rr % 4) != 3 else 1.0
    w0sin1 = w0sin0 * sgn[:, None]
    w0cos = np.concatenate([w0cos0, w0cos0], 1).astype(np.float32)   # [70,256]
    w0sin = np.concatenate([w0sin0, w0sin1], 1).astype(np.float32)
    w0t0 = np.concatenate([W0[0:6], b0[None, :]], 0)                 # [7,128]
    w0t1 = np.concatenate([W0[3:6], W0[0:3], b0[None, :]], 0)
    w0t = np.concatenate([w0t0, w0t1], 1).astype(np.float32)         # [7,256]

    asel = np.zeros((128, PPB * 77), np.float32)
    for p in range(PPB):
        for i in range(EMB):
            c = float(2.0 ** (i - 1))           # turns per unit x
            for d in range(3):
                asel[16 * p + d, p * 77 + i * 3 + d] = c
            for q in range(4):
                asel[16 * p + 3 + q, p * 77 + 30 + i * 4 + q] = c
        for c2 in range(6):
            asel[16 * p + 7 + c2, p * 77 + 70 + c2] = 1.0
        asel[16 * p + 13, p * 77 + 76] = 1.0

    w3pad = np.zeros((128, 32 * 32), np.float32)
    for j in range(32):
        w3pad[:, 32 * j + j] = W3[:, 0]
    return w0cos, w0sin, w0t, asel, w3pad


def _build_program(b3f):
    nc = bacc.Bacc("TRN2", target_bir_lowering=False, debug=False,
                   num_devices=NCORES)
    pos_d = nc.dram_tensor("pos", [BC, NN * 3], F32, kind="ExternalInput")
    quat_d = nc.dram_tensor("quat", [BC, NN * 4], F32, kind="ExternalInput")
    obj_d = nc.dram_tensor("obj", [BC, NN * 3], F32, kind="ExternalInput")
    asel_d = nc.dram_tensor("asel", [128, PPB * 77], F32, kind="ExternalInput")
    ident_d = nc.dram_tensor("ident", [128, 128], F32, kind="ExternalInput")
    w0cos_d = nc.dram_tensor("w0cos", [70, 256], F32, kind="ExternalInput")
    w0sin_d = nc.dram_tensor("w0sin", [70, 256], F32, kind="ExternalInput")
    w0t_d = nc.dram_tensor("w0t", [7, 256], F32, kind="ExternalInput")
    w1_d = nc.dram_tensor("w1", [128, 128], F32, kind="ExternalInput")
    w2_d = nc.dram_tensor("w2", [128, 128], F32, kind="ExternalInput")
    w3p_d = nc.dram_tensor("w3p", [128, 32 * 32], F32, kind="ExternalInput")
    b1_d = nc.dram_tensor("b1", [128, 1], F32, kind="ExternalInput")
    b2_d = nc.dram_tensor("b2", [128, 1], F32, kind="ExternalInput")
    out_d = nc.dram_tensor("out", [BC, NPAIR], F32, kind="ExternalOutput")

    with tile.TileContext(nc) as tc:
        with tc.tile_pool(name="cpool", bufs=1) as cpool, \
             tc.tile_pool(name="wpool", bufs=2) as wpool, \
             tc.tile_pool(name="psA", bufs=1, space=MS.PSUM) as psA, \
             tc.tile_pool(name="psB", bufs=2, space=MS.PSUM) as psB:

            def load_const(name, dram, shape):
                t = cpool.tile(shape, F32, tag=name)
                nc.sync.dma_start(t[:], dram[:])
                return t

            pos_sb = load_const("pos_sb", pos_d, [BC, NN * 3])
            quat_sb = load_const("quat_sb", quat_d, [BC, NN * 4])
            obj_sb = load_const("obj_sb", obj_d, [BC, NN * 3])
            asel_sb = load_const("asel_sb", asel_d, [128, PPB * 77])
            ident_sb = load_const("ident_sb", ident_d, [128, 128])
            w0cos_sb = load_const("w0cos_sb", w0cos_d, [70, 256])
            w0sin_sb = load_const("w0sin_sb", w0sin_d, [70, 256])
            w0t_sb = load_const("w0t_sb", w0t_d, [7, 256])
            w1_sb = load_const("w1_sb", w1_d, [128, 128])
            w2_sb = load_const("w2_sb", w2_d, [128, 128])
            w3p_sb = load_const("w3p_sb", w3p_d, [128, 32 * 32])
            b1_sb = load_const("b1_sb", b1_d, [128, 1])
            b2_sb = load_const("b2_sb", b2_d, [128, 1])

            negq = cpool.tile([BC, NN * 4], F32, tag="negq")
            nc.gpsimd.tensor_scalar_mul(negq[:], quat_sb[:], -1.0)

            seed = cpool.tile([BC, NPAIR * 16], F32, tag="seed")
            nc.vector.memset(seed[:], 1.0)   # col 13 of each pair must be 1.0
            seedT = cpool.tile([128, NBLK * 128], F32, tag="seedT")
            out_sb = cpool.tile([BC, NPAIR], F32, tag="out_sb")

            posR = pos_sb[:].rearrange("b (k d) -> b k d", d=3)
            quatR = quat_sb[:].rearrange("b (k c) -> b k c", c=4)
            objR = obj_sb[:].rearrange("b (k d) -> b k d", d=3)
            seedR = seed[:].rearrange("b (p c) -> b p c", c=16)

            # ---- Stage B: seed build (token-major), group = first node i ----
            pbase = 0
            for i in range(NN - 1):
                J = NN - 1 - i
                G = seedR[:, pbase:pbase + J, :]
                vj = quatR[:, i + 1:, 0:3]
                wj = quatR[:, i + 1:, 3:4]
                wi = quat_sb[:, i * 4 + 3:i * 4 + 4]

                nc.vector.tensor_sub(
                    G[:, :, 0:3], posR[:, i + 1:, :],
                    posR[:, i:i + 1, :].broadcast_to((BC, J, 3)))
                # lq.v = wi*vj - wj*vi + vj x vi
                nc.vector.tensor_scalar(G[:, :, 3:6], vj, wi, None, ALU.mult)
                for c in range(3):
                    gc = G[:, :, 3 + c:4 + c]
                    c1, c2 = (c + 1) % 3, (c + 2) % 3
                    nc.vector.scalar_tensor_tensor(
                        gc, wj, negq[:, i * 4 + c:i * 4 + c + 1], gc,
                        ALU.mult, ALU.add)
                    nc.vector.scalar_tensor_tensor(
                        gc, quatR[:, i + 1:, c1:c1 + 1],
                        quat_sb[:, i * 4 + c2:i * 4 + c2 + 1], gc,
                        ALU.mult, ALU.add)
                    nc.vector.scalar_tensor_tensor(
                        gc, quatR[:, i + 1:, c2:c2 + 1],
                        negq[:, i * 4 + c1:i * 4 + c1 + 1], gc,
                        ALU.mult, ALU.add)
                # lq.w = wi*wj + vi.vj
                gw = G[:, :, 6:7]
                nc.vector.tensor_scalar(gw, wj, wi, None, ALU.mult)
                for c in range(3):
                    nc.vector.scalar_tensor_tensor(
                        gw, quatR[:, i + 1:, c:c + 1],
                        quat_sb[:, i * 4 + c:i * 4 + c + 1], gw,
                        ALU.mult, ALU.add)
                nc.vector.tensor_copy(
                    G[:, :, 7:10], objR[:, i:i + 1, :].broadcast_to((BC, J, 3)))
                nc.vector.tensor_copy(G[:, :, 10:13], objR[:, i + 1:, :])
                pbase += J

            # ---- per-chunk: transpose, angles, trig, MLP ----
            zpack = None
            RS = 16                       # chunks per z-round
            for t in range(NBLK):
                rnd, tl = t // RS, t % RS
                TL = min(RS, NBLK - RS * rnd)
                if tl == 0:
                    zpack = psA.tile([96, NH], F32, tag="zpack")

                tp = psA.tile([128, 128], F32, tag="tp")
                nc.tensor.transpose(tp[:], seed[:, t * 128:(t + 1) * 128],
                                    ident_sb[:])
                nc.vector.tensor_copy(seedT[:, t * 128:(t + 1) * 128], tp[:])

                u_ps = psA.tile([77, N1], F32, tag="u_ps")
                for p in range(PPB):
                    nc.tensor.matmul(
                        u_ps[:, p * BC:(p + 1) * BC],
                        asel_sb[:, p * 77:(p + 1) * 77],
                        seedT[:, t * 128:(t + 1) * 128],
                        start=True, stop=True)

                u_sb = wpool.tile([77, N1], F32, tag="u_sb")
                nc.vector.tensor_copy(u_sb[:], u_ps[:])
                xt = wpool.tile([7, N1], F32, tag="xt")
                nc.sync.dma_start(xt[:], u_sb[70:77, :])

                ua = u_sb[0:70, :]
                r = wpool.tile([70, N1], F32, tag="r")
                nc.gpsimd.tensor_scalar_add(r[:], ua, MAGIC)
                nc.gpsimd.tensor_scalar_sub(r[:], r[:], MAGIC)
                f = wpool.tile([70, N1], F32, tag="f")
                nc.gpsimd.tensor_sub(f[:], ua, r[:])
                r3 = wpool.tile([70, N1], F32, tag="r3")
                nc.gpsimd.tensor_scalar(r3[:], f[:], 0.25, None, ALU.is_ge)
                f2c = wpool.tile([70, N1], F32, tag="f2c")
                nc.gpsimd.tensor_scalar_add(f2c[:], f[:], 0.25)
                nc.gpsimd.tensor_sub(f2c[:], f2c[:], r3[:])

                xsin = wpool.tile([70, N1], F32, tag="xsin")
                nc.scalar.activation(xsin[:], f[:], AF.Sin, scale=TWO_PI)
                xcos = wpool.tile([70, N1], F32, tag="xcos")
                nc.scalar.activation(xcos[:], f2c[:], AF.Sin, scale=TWO_PI)

                for h in range(2):
                    cs = slice(h * NH, (h + 1) * NH)
                    for d in range(2):
                        ds = slice(d * 128, (d + 1) * 128)
                        h0p = psB.tile([128, NH], F32, tag="h0p")
                        nc.tensor.matmul(h0p[:], w0cos_sb[:, ds], xcos[:, cs],
                                         start=True, stop=False)
                        nc.tensor.matmul(h0p[:], w0sin_sb[:, ds], xsin[:, cs],
                                         start=False, stop=False)
                        nc.tensor.matmul(h0p[:], w0t_sb[:, ds], xt[:, cs],
                                         start=False, stop=True)
                        h0 = wpool.tile([128, NH], F32, tag="h0")
                        nc.vector.tensor_scalar_max(h0[:], h0p[:], 0.0)

                        h1p = psA.tile([128, NH], F32, tag="h1p")
                        nc.tensor.matmul(h1p[:], w1_sb[:], h0[:],
                                         start=True, stop=True)
                        h1 = wpool.tile([128, NH], F32, tag="h1")
                        if d == 0:
                            nc.vector.tensor_scalar(h1[:], h1p[:], b1_sb[:],
                                                    0.0, ALU.add, ALU.max)
                        else:
                            nc.scalar.activation(h1[:], h1p[:], AF.Relu,
                                                 bias=b1_sb[:])

                        h2p = psA.tile([128, NH], F32, tag="h2p")
                        nc.tensor.matmul(h2p[:], w2_sb[:], h1[:],
                                         start=True, stop=True)
                        h2 = wpool.tile([128, NH], F32, tag="h2")
                        if d == 0 and h == 0:
                            nc.vector.tensor_scalar(h2[:], h2p[:], b2_sb[:],
                                                    0.0, ALU.add, ALU.max)
                        else:
                            nc.scalar.activation(h2[:], h2p[:], AF.Relu,
                                                 bias=b2_sb[:])

                        j = 2 * tl + h
                        nc.tensor.matmul(
                            zpack[64 * d:64 * d + 32, :],
                            w3p_sb[:, 32 * j:32 * j + 32], h2[:],
                            start=(j == 0), stop=(h == 1 and tl == TL - 1))

                # ---- round flush: tanh, mean over dirs, scatter to out ----
                if tl == TL - 1:
                    CL = TL
                    zS = wpool.tile([96, NH], F32, tag="zS")
                    nc.vector.tensor_copy(zS[:], zpack[:])
                    outv = out_sb[:].rearrange("b (q g) -> b q g", g=4)
                    for g in range(4):
                        ztP = psA.tile([128, 96], F32, tag="tp")
                        nc.tensor.transpose(ztP[:], zS[:, g * 128:(g + 1) * 128],
                                            ident_sb[0:96, 0:96])
                        ztS = wpool.tile([128, 96], F32, tag="ztS")
                        nc.scalar.activation(ztS[:], ztP[:], AF.Tanh, bias=b3f)
                        ztmp = wpool.tile([128, 32], F32, tag="ztmp")
                        nc.vector.tensor_tensor(
                            ztmp[:, 0:2 * CL], ztS[:, 0:2 * CL],
                            ztS[:, 64:64 + 2 * CL], ALU.add)
                        vv = outv[:, 32 * rnd:32 * rnd + 2 * CL, g:g + 1]
                        tmpv = ztmp[:].rearrange("b (q one) -> b q one",
                                                 one=1)[:, 0:2 * CL, :]
                        nc.vector.tensor_scalar(vv, tmpv, PEN * 0.5, None,
                                                ALU.mult)
            nc.sync.dma_start(out_d[:], out_sb[:])
    nc.compile()
    return nc


_PROGRAM_CACHE = {}


def _get_program(b3f):
    if b3f not in _PROGRAM_CACHE:
        _PROGRAM_CACHE[b3f] = _build_program(b3f)
    return _PROGRAM_CACHE[b3f]


def make_in_maps(obj_type, gparam, pos, quat, W0, b0, W1, b1, W2, b2, W3, b3,
                 **_unused):
    del gparam
    W0 = np.asarray(W0, np.float32)
    b0 = np.asarray(b0, np.float32)
    W3 = np.asarray(W3, np.float32).reshape(128, 1)
    w0cos, w0sin, w0t, asel, w3pad = _host_operands(W0, b0, W3)
    shared = {
        "asel": asel,
        "ident": np.eye(128, dtype=np.float32),
        "w0cos": w0cos,
        "w0sin": w0sin,
        "w0t": w0t,
        "w1": np.ascontiguousarray(W1, np.float32),
        "w2": np.ascontiguousarray(W2, np.float32),
        "w3p": w3pad,
        "b1": np.asarray(b1, np.float32).reshape(128, 1),
        "b2": np.asarray(b2, np.float32).reshape(128, 1),
    }
    obj_type = np.asarray(obj_type, np.float32)
    pos = np.asarray(pos, np.float32)
    quat = np.asarray(quat, np.float32)
    in_maps = []
    for c in range(NCORES):
        s = slice(c * BC, (c + 1) * BC)
        in_maps.append({
            "pos": np.ascontiguousarray(pos[s]).reshape(BC, NN * 3),
            "quat": np.ascontiguousarray(quat[s]).reshape(BC, NN * 4),
            "obj": np.ascontiguousarray(obj_type[s]).reshape(BC, NN * 3),
            **shared,
        })
    return in_maps


def kernel(**inputs):
    in_maps = make_in_maps(**inputs)
    nc = _get_program(float(np.float32(inputs["b3"]).reshape(-1)[0]))
    res = run_bass_kernel_spmd(nc, in_maps, core_ids=list(range(NCORES)))
    out = np.concatenate([res.results[c]["out"] for c in range(NCORES)], axis=0)
    return np.ascontiguousarray(out, np.float32)
